# revision 6
# baseline (speedup 1.0000x reference)
"""MHA kernel for Trainium2, 8-core tensor-parallel (2 heads per core).

Problem (hardcoded): x [2, 2048, 1024] fp32, Wq/Wk/Wv/Wo [1024, 1024],
bq/bk/bv/bo [1024], H=16 heads, DH=64.  out = MHA(x).

Sharding: heads split 8 ways (2 heads = 128 proj columns per core).
Each core computes its heads' attention and a partial row-parallel
output projection; the host sums the 8 partials and adds the
closed-form bias terms (bv @ Wo + bo).

v2 design (per core, all matmuls bf16 in / fp32 accumulate — fp8 was
measured to cost ~2-4e-2 rel err because attention output is a
near-uniform average of zero-mean V, so quantization noise does not
average out):
  - scores S^T per ktile [128, 2x512] via K=64 matmuls, two heads
    packed on PE row groups 0-63 / 64-127.
  - exp on ScalarE (the kernel bottleneck, ~1 elem/lane/cycle,
    FD=1024 per call), P in bf16.
  - AV with an extra ones-column in V (PSUM row 64) for the softmax
    denominator; reciprocal via DVE reciprocal_approx_fast (ScalarE
    stays exp-only); broadcast via GPSIMD partition_broadcast.
  - QKV work (K/Q chunks, token-major V tiles) is interleaved into the
    attention combo streams: ScalarE starts exp'ing ~10us in and the
    PE always has independent work between dependency stalls.
"""

import numpy as np
import ml_dtypes

D = 1024
T = 4096          # B*S tokens
S = 2048
B = 2
NH = 2            # heads per core
DH = 64
NCORES = 8
NKT = S // 128    # 16 key tiles per batch
NQC = S // 512    # 4 query chunks per batch
SLOT = DH + 1     # 65: V columns + ones column
SCALE = 0.125     # 1/sqrt(DH)

_CACHE = {}


def _build_nc():
    import concourse.bacc as bacc
    import concourse.mybir as mybir
    import concourse.tile as tile

    dt = mybir.dt
    f32, bf16 = dt.float32, dt.bfloat16
    MULT = mybir.AluOpType.mult
    EXP = mybir.ActivationFunctionType.Exp

    nc = bacc.Bacc("TRN2", target_bir_lowering=False, debug=False,
                   num_devices=NCORES)

    xT_d = nc.dram_tensor("xT", [D, T], bf16, kind="ExternalInput")
    wq_d = nc.dram_tensor("wq", [D, 128], bf16, kind="ExternalInput")
    wk_d = nc.dram_tensor("wk", [D, 128], bf16, kind="ExternalInput")
    wv_d = nc.dram_tensor("wv", [D, 128], bf16, kind="ExternalInput")
    wo_d = nc.dram_tensor("wo", [128, D], bf16, kind="ExternalInput")
    bq_d = nc.dram_tensor("bq", [128, 1], f32, kind="ExternalInput")
    bk_d = nc.dram_tensor("bk", [128, 1], f32, kind="ExternalInput")
    outp = nc.dram_tensor("outp", [T, D], f32, kind="ExternalOutput")

    with tile.TileContext(nc) as tc:
        with (
            tc.tile_pool(name="persist", bufs=1) as pp,
            tc.tile_pool(name="pt", bufs=2) as ptp,
            tc.tile_pool(name="onorm", bufs=2) as onp,
            tc.tile_pool(name="oraw", bufs=3) as orp,
            tc.tile_pool(name="dn", bufs=4) as dnp,
            tc.tile_pool(name="rb", bufs=3) as rbp,
            tc.tile_pool(name="outsb", bufs=4) as osp,
        ):
            # ---- persistent SBUF tensors ----
            # preload the exp table set so the first real exp call does
            # not pay the ~2.7us ACT_TABLE_LOAD on the critical path
            warm = pp.tile([1, 8], f32, tag="warm")
            nc.vector.memset(warm[:, :], 0.0)
            nc.scalar.activation(warm[:, :], warm[:, :], EXP)

            w8 = {}
            w_tiles = {}
            for nm, wd in (("q", wq_d), ("k", wk_d), ("v", wv_d)):
                w = pp.tile([128, 8 * 128], bf16, tag=f"w8{nm}")
                w_tiles[nm] = (w, wd)
                w8[nm] = w.rearrange("p (d m) -> p d m", d=8)
            wo = pp.tile([128, D], bf16, tag="wo")
            bq = pp.tile([128, 1], f32, tag="bq")
            bk = pp.tile([128, 1], f32, tag="bk")
            xt = pp.tile([128, 8 * T], bf16, tag="xt")
            x3 = xt.rearrange("p (d c) -> p d c", d=8)

            def dma_w(nm):
                w, wd = w_tiles[nm]
                nc.sync.dma_start(
                    out=w.rearrange("p (d m) -> p d m", d=8),
                    in_=wd.ap().rearrange("(d p) m -> p d m", p=128),
                )

            def dma_x(nck):
                cs = slice(nck * 512, (nck + 1) * 512)
                for d in range(8):
                    nc.sync.dma_start(
                        out=x3[:, d, cs],
                        in_=xT_d.ap()[d * 128:(d + 1) * 128, cs])

            # DMA order: what the lead-in needs first.
            nc.sync.dma_start(out=bq[:, :], in_=bq_d.ap()[:, :])
            nc.sync.dma_start(out=bk[:, :], in_=bk_d.ap()[:, :])
            dma_w("k")
            dma_w("q")
            dma_x(0)
            dma_w("v")
            dma_x(1)
            nc.sync.dma_start(out=wo[:, :], in_=wo_d.ap()[:, :])
            for nck in range(2, T // 512):
                dma_x(nck)

            qt = pp.tile([128, T], bf16, tag="qt")
            kt = pp.tile([128, T], bf16, tag="kt")

            v4 = []
            for b in range(B):
                v = pp.tile([128, NH * NKT * SLOT], bf16, tag=f"v4_{b}")
                vr = v.rearrange("p (h k c) -> p h k c", h=NH, k=NKT)
                nc.vector.memset(vr[:, :, :, DH:DH + 1], 1.0)
                v4.append(vr)

            with (
                tc.tile_pool(name="st_ps", bufs=2, space="PSUM") as stp,
                tc.tile_pool(name="av_ps", bufs=2, space="PSUM") as avp,
                tc.tile_pool(name="mm_ps", bufs=2, space="PSUM") as mmp,
            ):
                # ---------- building blocks ----------
                def proj_chunk(proj_sb, w3, b_sb, nck):
                    """One 512-col chunk of Q^T/K^T: 8 K=128 matmuls."""
                    cs = slice(nck * 512, (nck + 1) * 512)
                    ps = mmp.tile([128, 512], f32, tag="mm",
                                  name=f"proj{nck}")
                    for d in range(8):
                        nc.tensor.matmul(
                            ps[:, :], w3[:, d, :], x3[:, d, cs],
                            start=(d == 0), stop=(d == 7),
                        )
                    nc.vector.tensor_scalar_add(proj_sb[:, cs], ps[:, :],
                                                b_sb[:, :])

                def v_tile(b, k):
                    """Token-major V tile [128 tok, 128 vdim] -> v4 bf16."""
                    t0 = b * S + k * 128
                    ps = mmp.tile([128, 512], f32, tag="mm", name=f"v{b}_{k}")
                    for d in range(8):
                        nc.tensor.matmul(
                            ps[:, 0:128], x3[:, d, t0:t0 + 128],
                            w8["v"][:, d, :],
                            start=(d == 0), stop=(d == 7),
                        )
                    nc.vector.tensor_copy(
                        v4[b][:, :, k, 0:DH],
                        ps[:, 0:128].rearrange("p (h c) -> p h c", h=NH),
                    )

                def emit_outproj(q0, onorm):
                    for s4 in range(4):
                        for jc in range(2):
                            op = mmp.tile([128, 512], f32, tag="mm",
                                          name=f"op{q0}_{s4}_{jc}")
                            nc.tensor.matmul(
                                op[:, :], onorm[:, s4 * 128:(s4 + 1) * 128],
                                wo[:, jc * 512:(jc + 1) * 512],
                                start=True, stop=True,
                            )
                            osb = osp.tile([128, 512], f32, tag="outsb",
                                           name=f"osb{q0}_{s4}_{jc}")
                            nc.vector.tensor_copy(osb[:, :], op[:, :])
                            r0 = q0 + s4 * 128
                            nc.sync.dma_start(
                                out=outp.ap()[r0:r0 + 128,
                                              jc * 512:(jc + 1) * 512],
                                in_=osb[:, :],
                            )

                # foreign-work queues injected into combo pair slots.
                def mk_slots():
                    return [[] for _ in range(8)]

                inject = {}
                # combo (b,0) is self-feeding: K(b) c1-3 land just before
                # the score ktiles that need them, V(b) tiles just before
                # their AV pair, Q(b) c1-3 before the later combos.  This
                # splits the QKV matmul load evenly between the two
                # batches' combo windows so ScalarE (the bottleneck) is
                # never starved by a PE backlog.
                for b in range(B):
                    sl = mk_slots()
                    sl[0].append(lambda b=b: proj_chunk(kt, w8["k"], bk,
                                                        4 * b + 1))
                    sl[1].append(lambda b=b: proj_chunk(kt, w8["k"], bk,
                                                        4 * b + 2))
                    sl[2].append(lambda b=b: proj_chunk(kt, w8["k"], bk,
                                                        4 * b + 3))
                    for i in range(8):
                        sl[i].append(lambda b=b, i=i: v_tile(b, 2 * i))
                        sl[i].append(lambda b=b, i=i: v_tile(b, 2 * i + 1))
                    sl[3].append(lambda b=b: proj_chunk(qt, w8["q"], bq,
                                                        4 * b + 1))
                    sl[4].append(lambda b=b: proj_chunk(qt, w8["q"], bq,
                                                        4 * b + 2))
                    sl[5].append(lambda b=b: proj_chunk(qt, w8["q"], bq,
                                                        4 * b + 3))
                    inject[(b, 0)] = sl
                # b1's c0 chunks must complete before combo (1,0) starts.
                sl = mk_slots()
                sl[0].append(lambda: proj_chunk(kt, w8["k"], bk, 4))
                inject[(0, 2)] = sl
                sl = mk_slots()
                sl[0].append(lambda: proj_chunk(qt, w8["q"], bq, 4))
                inject[(0, 3)] = sl

                # ---------- lead-in ----------
                proj_chunk(kt, w8["k"], bk, 0)
                proj_chunk(qt, w8["q"], bq, 0)

                # ---------- attention combos ----------
                pending = None
                for b in range(B):
                    for qc in range(NQC):
                        q0 = b * S + qc * 512
                        slots = inject.get((b, qc), mk_slots())
                        pt = ptp.tile([128, NH * NKT * 512], bf16, tag="pt",
                                      name=f"pt{b}_{qc}")
                        pt4 = pt.rearrange("p (h k q) -> p h k q",
                                           h=NH, k=NKT)
                        onorm = onp.tile([128, 512], bf16, tag="onorm",
                                         name=f"onorm{b}_{qc}")
                        avh = [avp.tile([128, 512], f32, tag="av",
                                        name=f"av{b}_{qc}_{h}")
                               for h in range(NH)]
                        emitted_op = False
                        for i in range(8):
                            for j in range(2):
                                kti = 2 * i + j
                                k0 = b * S + kti * 128
                                st = stp.tile([128, 1024], f32, tag="st")
                                for h in range(NH):
                                    hp = h * DH
                                    nc.tensor.matmul(
                                        st[:, h * 512:(h + 1) * 512],
                                        kt[hp:hp + DH, k0:k0 + 128],
                                        qt[hp:hp + DH, q0:q0 + 512],
                                        start=True, stop=True,
                                    )
                                nc.scalar.activation(
                                    pt4[:, :, kti, :], st[:, :], EXP,
                                    scale=SCALE,
                                )
                            for fn in slots[i]:
                                fn()
                            for h in range(NH):
                                for j in range(2):
                                    kti = 2 * i + j
                                    nc.tensor.matmul(
                                        avh[h][0:SLOT, :],
                                        v4[b][:, h, kti, 0:SLOT],
                                        pt4[:, h, kti, :],
                                        start=(kti == 0), stop=(kti == 15),
                                    )
                            # previous combo's output projection mid-combo
                            if i == 3 and pending is not None:
                                emit_outproj(*pending)
                                pending = None
                                emitted_op = True

                        # softmax normalization chain
                        oraw = [orp.tile([DH, 512], f32, tag="oraw",
                                         name=f"oraw{b}_{qc}_{h}")
                                for h in range(NH)]
                        for h in range(NH):
                            nc.vector.tensor_copy(oraw[h][:, :],
                                                  avh[h][0:DH, :])
                            denom = dnp.tile([1, 512], f32, tag="dn",
                                             name=f"dn{b}_{qc}_{h}")
                            recip = dnp.tile([1, 512], f32, tag="rc",
                                             name=f"rc{b}_{qc}_{h}")
                            nc.vector.tensor_copy(denom[:, :],
                                                  avh[h][DH:DH + 1, :])
                            nc.vector.reciprocal_approx_fast(
                                out=recip[:, :], in_=denom[:, :])
                            hp = h * DH
                            rb = rbp.tile([DH, 512], f32, tag="rb",
                                          name=f"rb{b}_{qc}_{h}")
                            nc.gpsimd.partition_broadcast(
                                rb[:, :], recip[:, :])
                            nc.vector.tensor_tensor(
                                onorm[hp:hp + DH, :], oraw[h][:, :],
                                rb[:, :], op=MULT,
                            )
                        if pending is not None and not emitted_op:
                            emit_outproj(*pending)
                        pending = (q0, onorm)
                emit_outproj(*pending)

    nc.compile()
    return nc


def _prep_inputs(x, Wq, bq, Wk, bk, Wv, bv, Wo, bo):
    bf16 = ml_dtypes.bfloat16
    xT = np.ascontiguousarray(
        np.asarray(x, dtype=np.float32).reshape(T, D).T).astype(bf16)
    in_maps = []
    for c in range(NCORES):
        cs = slice(c * 128, (c + 1) * 128)
        in_maps.append({
            "xT": xT,
            "wq": np.ascontiguousarray(Wq[:, cs]).astype(bf16),
            "wk": np.ascontiguousarray(Wk[:, cs]).astype(bf16),
            "wv": np.ascontiguousarray(Wv[:, cs]).astype(bf16),
            "wo": np.ascontiguousarray(Wo[cs, :]).astype(bf16),
            "bq": np.ascontiguousarray(bq[cs]).reshape(128, 1).astype(np.float32),
            "bk": np.ascontiguousarray(bk[cs]).reshape(128, 1).astype(np.float32),
        })
    return in_maps


def kernel(x, Wq, bq, Wk, bk, Wv, bv, Wo, bo, _trace=False, _results=None):
    from concourse.bass_utils import run_bass_kernel_spmd

    x = np.asarray(x); Wq = np.asarray(Wq); Wk = np.asarray(Wk)
    Wv = np.asarray(Wv); Wo = np.asarray(Wo)
    bq = np.asarray(bq); bk = np.asarray(bk); bv = np.asarray(bv)
    bo = np.asarray(bo)

    if "nc" not in _CACHE:
        _CACHE["nc"] = _build_nc()
    nc = _CACHE["nc"]

    in_maps = _prep_inputs(x, Wq, bq, Wk, bk, Wv, bv, Wo, bo)
    res = run_bass_kernel_spmd(
        nc, in_maps, core_ids=list(range(NCORES)), trace=_trace)
    if _results is not None:
        _results.append(res)

    acc = np.zeros((T, D), dtype=np.float32)
    for c in range(NCORES):
        acc += np.asarray(res.results[c]["outp"], dtype=np.float32)
    acc += bv.astype(np.float32) @ Wo.astype(np.float32) + bo.astype(np.float32)
    return acc.reshape(B, S, D)
